# revision 1
# baseline (speedup 1.0000x reference)
"""Trainium2 Bass kernel for nn_CustomEmbeddingRegularizer.

Computes  RATE * (sum(x^2) - sum_i mean_{j in nbr(i)} x_i . x_j)
        = RATE * (sum(x^2) - sum_e w_e * (x[src_e] . x[dst_e])),  w_e = 1/deg(src_e)

Distribution: edges sharded 8 ways (contiguous slices of the src-sorted edge
list). Each core gathers its edges' src rows from a per-core 16K-row table
slice (sorted src spans ~N/8 rows) and dst rows from the replicated full
table via int16 dma_gather against four 32K-row base windows (edges are
stable-partitioned by dst window on the host; the per-edge weight travels
with the permutation so ordering never matters). Per-edge dots and the
weighted reduction run on DVE; sum(x^2) of a disjoint N/8 row slice runs on
ACT. Host sums the 8 [128,2] partials.
"""

import numpy as np

import concourse.bacc as bacc
import concourse.bass as bass
import concourse.mybir as mybir
from concourse.tile import TileContext
from concourse.bass_utils import run_bass_kernel_spmd

RATE = 4 * 0.01
N_CORES = 8
P = 128
D = 128
BUCKET = 32768          # int16-addressable row window for the dst gather
SRC_SLICE = 16384       # per-core src-slice rows (covers max src span per shard)
B = 4096                # edges per batch (= one dma_gather)
C = B // P              # edge columns per partition

_CACHE = {}


def _build(N, NB, sched):
    """Compile the SPMD kernel: NB batches, sched[b] = dst bucket id."""
    nc = bacc.Bacc("TRN2", target_bir_lowering=False, num_swdge_queues=4)
    t_table = nc.dram_tensor("table", [N, D], mybir.dt.float32, kind="ExternalInput")
    t_src_slice = nc.dram_tensor("src_slice", [SRC_SLICE, D], mybir.dt.float32,
                                 kind="ExternalInput")
    t_sq_slice = nc.dram_tensor("sq_slice", [N // N_CORES, D], mybir.dt.float32,
                                kind="ExternalInput")
    t_idx_s = nc.dram_tensor("idx_s", [NB, P, B // 16], mybir.dt.int16,
                             kind="ExternalInput")
    t_idx_d = nc.dram_tensor("idx_d", [NB, P, B // 16], mybir.dt.int16,
                             kind="ExternalInput")
    t_w = nc.dram_tensor("w", [NB, P, C], mybir.dt.float32, kind="ExternalInput")
    t_out = nc.dram_tensor("out", [P, 2], mybir.dt.float32, kind="ExternalOutput")

    FSQ = (N // N_CORES) * D // P    # sumsq free elems per partition

    NSQ = 4
    FCH = FSQ // NSQ

    with TileContext(nc) as tc:
        with (
            tc.tile_pool(name="big", bufs=2) as big,
            tc.tile_pool(name="small", bufs=3) as small,
            tc.tile_pool(name="sqp", bufs=2) as sqp,
            tc.tile_pool(name="accp", bufs=1) as accp,
        ):
            acc = accp.tile([P, 1], mybir.dt.float32, tag="acc")
            nc.vector.memset(acc[:], 0.0)
            sq = accp.tile([P, 1], mybir.dt.float32, tag="sq")
            nc.vector.memset(sq[:], 0.0)

            sq_flat = t_sq_slice[:].rearrange("a b -> (a b)").rearrange(
                "(p f) -> p f", p=P)
            for ch in range(NSQ):
                sl_tile = sqp.tile([P, FCH], mybir.dt.float32, tag="sl")
                nc.sync.dma_start(out=sl_tile[:],
                                  in_=sq_flat[:, ch * FCH:(ch + 1) * FCH])
                sq_scratch = sqp.tile([P, FCH], mybir.dt.float32, tag="sqs")
                sqc = sqp.tile([P, 1], mybir.dt.float32, tag="sqc")
                nc.scalar.activation(out=sq_scratch[:], in_=sl_tile[:],
                                     func=mybir.ActivationFunctionType.Square,
                                     accum_out=sqc[:])
                nc.vector.tensor_tensor(out=sq[:], in0=sq[:], in1=sqc[:],
                                        op=mybir.AluOpType.add)

            q = 0
            for b in range(NB):
                base = sched[b] * BUCKET
                dst_src_ap = t_table[base:min(base + BUCKET, N)]

                xs = big.tile([P, C, D], mybir.dt.float32, tag="xs")
                xd = big.tile([P, C, D], mybir.dt.float32, tag="xd")
                prod = big.tile([P, C, D], mybir.dt.float32, tag="prod")
                ist = small.tile([P, B // 16], mybir.dt.int16, tag="ist")
                idt = small.tile([P, B // 16], mybir.dt.int16, tag="idt")
                wt = small.tile([P, C], mybir.dt.float32, tag="wt")
                dots = small.tile([P, C], mybir.dt.float32, tag="dots")
                wd = small.tile([P, C], mybir.dt.float32, tag="wd")
                bs = small.tile([P, 1], mybir.dt.float32, tag="bs")

                nc.sync.dma_start(out=ist[:], in_=t_idx_s[b])
                nc.sync.dma_start(out=idt[:], in_=t_idx_d[b])
                nc.sync.dma_start(out=wt[:], in_=t_w[b])

                # split each gather across two SWDGE queues: the Q7
                # descriptor-generation rate is the bottleneck and queues
                # process in parallel (wrapped idx layout splits cleanly:
                # idx j -> [j%16, j//16], so halves are column ranges)
                H = B // 2
                HC = C // 2
                for half in range(2):
                    cs = slice(half * (H // 16), (half + 1) * (H // 16))
                    nc.gpsimd.dma_gather(
                        out_ap=xs[:, half * HC:(half + 1) * HC, :],
                        in_ap=t_src_slice[:], idxs_ap=ist[:, cs],
                        num_idxs=H, num_idxs_reg=H, elem_size=D,
                        single_packet=False, queue_num=q % 4)
                    q += 1
                for half in range(2):
                    cs = slice(half * (H // 16), (half + 1) * (H // 16))
                    nc.gpsimd.dma_gather(
                        out_ap=xd[:, half * HC:(half + 1) * HC, :],
                        in_ap=dst_src_ap, idxs_ap=idt[:, cs],
                        num_idxs=H, num_idxs_reg=H, elem_size=D,
                        single_packet=False, queue_num=q % 4)
                    q += 1

                nc.vector.tensor_tensor(out=prod[:], in0=xs[:], in1=xd[:],
                                        op=mybir.AluOpType.mult)
                nc.vector.tensor_reduce(out=dots[:], in_=prod[:],
                                        axis=mybir.AxisListType.X,
                                        op=mybir.AluOpType.add)
                nc.vector.tensor_tensor(out=wd[:], in0=dots[:], in1=wt[:],
                                        op=mybir.AluOpType.mult)
                nc.vector.tensor_reduce(out=bs[:], in_=wd[:],
                                        axis=mybir.AxisListType.X,
                                        op=mybir.AluOpType.add)
                nc.vector.tensor_tensor(out=acc[:], in0=acc[:], in1=bs[:],
                                        op=mybir.AluOpType.add)

            out_t = accp.tile([P, 2], mybir.dt.float32, tag="out")
            nc.vector.tensor_copy(out=out_t[:, 0:1], in_=acc[:])
            nc.vector.tensor_copy(out=out_t[:, 1:2], in_=sq[:])
            nc.sync.dma_start(out=t_out[:], in_=out_t[:])
    nc.compile()
    return nc


def _wrap_idx(a):
    """[B] int16 -> [128, B//16] wrapped (j -> [j%16, j//16]) + replicated x8."""
    blk = a.reshape(B // 16, 16).T
    return np.tile(blk, (8, 1))


def kernel(inputs, edge_src, edge_dst):
    x = np.ascontiguousarray(np.asarray(inputs, dtype=np.float32))
    src = np.asarray(edge_src)
    dst = np.asarray(edge_dst)
    N = x.shape[0]
    E = src.shape[0]
    Ec = E // N_CORES
    assert E % N_CORES == 0 and x.shape[1] == D and N % N_CORES == 0

    src32 = src.astype(np.int64)
    dst32 = dst.astype(np.int64)
    deg = np.bincount(src32, minlength=N)
    w_all = (1.0 / np.maximum(deg, 1))[src32].astype(np.float32)

    n_buckets = (N + BUCKET - 1) // BUCKET

    # per-core, per-bucket edge lists (edge order within a core is free: the
    # weight travels with the edge)
    per_core = []
    for k in range(N_CORES):
        lo, hi = k * Ec, (k + 1) * Ec
        s = src32[lo:hi]
        d = dst32[lo:hi]
        w = w_all[lo:hi]
        b0 = int(s.min())
        span = int(s.max()) - b0 + 1
        if span > SRC_SLICE:
            raise ValueError(f"src span {span} exceeds SRC_SLICE {SRC_SLICE}")
        sl = s - b0                      # local src idx
        g = d >> 15                      # dst bucket (32768 = 2^15)
        order = np.argsort(g, kind="stable")
        per_core.append((b0, sl[order], d[order] - (g[order] << 15),
                         w[order], np.bincount(g, minlength=n_buckets)))

    counts = np.stack([pc[4] for pc in per_core])          # [cores, buckets]
    gmax = counts.max(axis=0)                              # padded per-bucket size
    nb_g = [int(-(-int(m) // B)) for m in gmax]            # batches per bucket
    NB = sum(nb_g)
    sched = []
    for gidx, nb in enumerate(nb_g):
        sched += [gidx] * nb

    key = (N, NB, tuple(sched))
    if key not in _CACHE:
        _CACHE[key] = _build(N, NB, sched)
    nc = _CACHE[key]

    in_maps = []
    for k in range(N_CORES):
        b0, sl, dl, w, cnt = per_core[k]
        # assemble padded per-bucket streams in schedule order
        idx_s = np.zeros((NB, B), dtype=np.int16)
        idx_d = np.zeros((NB, B), dtype=np.int16)
        wv = np.zeros((NB, B), dtype=np.float32)
        pos = 0
        bslot = 0
        for gidx, nb in enumerate(nb_g):
            n = int(cnt[gidx])
            seg_s = sl[pos:pos + n]
            seg_d = dl[pos:pos + n]
            seg_w = w[pos:pos + n]
            pos += n
            flat_s = np.zeros(nb * B, dtype=np.int16)
            flat_d = np.zeros(nb * B, dtype=np.int16)
            flat_w = np.zeros(nb * B, dtype=np.float32)
            flat_s[:n] = seg_s
            flat_d[:n] = seg_d
            flat_w[:n] = seg_w
            idx_s[bslot:bslot + nb] = flat_s.reshape(nb, B)
            idx_d[bslot:bslot + nb] = flat_d.reshape(nb, B)
            wv[bslot:bslot + nb] = flat_w.reshape(nb, B)
            bslot += nb

        idx_s_w = np.stack([_wrap_idx(a) for a in idx_s])
        idx_d_w = np.stack([_wrap_idx(a) for a in idx_d])
        # w layout: edge j -> (partition j%128, col j//128)
        w_t = wv.reshape(NB, C, P).transpose(0, 2, 1).copy()

        src_slice = np.zeros((SRC_SLICE, D), dtype=np.float32)
        avail = min(SRC_SLICE, N - b0)
        src_slice[:avail] = x[b0:b0 + avail]
        sq_slice = x[k * (N // N_CORES):(k + 1) * (N // N_CORES)]

        in_maps.append({
            "table": x,
            "src_slice": src_slice,
            "sq_slice": np.ascontiguousarray(sq_slice),
            "idx_s": idx_s_w,
            "idx_d": idx_d_w,
            "w": w_t,
        })

    res = run_bass_kernel_spmd(nc, in_maps, core_ids=list(range(N_CORES)))
    neighbor = 0.0
    sumsq = 0.0
    for k in range(N_CORES):
        out = res.results[k]["out"].astype(np.float64)
        neighbor += out[:, 0].sum()
        sumsq += out[:, 1].sum()
    return np.float32(RATE * (sumsq - neighbor))

